# revision 6
# baseline (speedup 1.0000x reference)
"""Trainium2 Bass kernel for ContrastiveTokenRepresentations.

Computes: sims = onehot @ protos.T (a row gather), then hard gumbel-softmax
(straight-through) over the 32 prototype logits.  The forward output is
numerically y_hard - y_soft + y_soft, elementwise in f32.

Strategy (data-parallel over 8 cores):
  - shard the [8192, 50257] onehot rows as 1024 rows/core
  - per core, stream the shard through SBUF in [128, 8192] tiles; VectorE
    multiplies each tile in place by a FIXED 1-based local iota, ScalarE
    row-sum-accumulates the product: per chunk this yields f_c = 1+j_local
    if the row's single 1.0 is in chunk c, else 0 (exact in f32).
  - per row-tile, v* = sum_c (f_c + c*W - 1) * min(f_c, 1) recovers the
    global token index; indirect-DMA gathers protoT[v*] -> sims [128, 32]
  - small softmax + straight-through one-hot tail per 128-row tile; with a
    row-outer loop the tail overlaps the next row's DMA stream
"""

import numpy as np

import concourse.bass as bass
import concourse.tile as tile
from concourse import mybir
from concourse.bass_utils import run_bass_kernel_spmd

B, S, V, NB = 4, 2048, 50257, 32
TEMPERATURE = 0.07
N_CORES = 8
R = (B * S) // N_CORES  # rows per core (1024)
P = 128                 # SBUF partitions
RT = R // P             # row tiles per core (8)
W = 8192                # column chunk width
NCH = (V + W - 1) // W  # 7 chunks (last = 1105 wide)

# test.py hooks: set TRACE=True before calling kernel() to capture an NTFF
# profile; LAST_RESULT then holds the BassKernelResults (exec_time_ns etc).
TRACE = False
LAST_RESULT = None

_PROGRAM = None

f32 = mybir.dt.float32


def _legalize_sync(nc):
    """This toolchain's walrus codegen allows exactly one sync-wait and one
    sync-update slot per instruction, but Tile emits instructions carrying
    several (e.g. the kernel-tail Drain waits on every DMA queue). Split the
    extras into single-sync NoOps: waits go on NoOps inserted just before the
    instruction (same engine, so program order preserves semantics), updates
    on NoOps just after."""

    def fix_block(bb):
        new = []
        changed = False
        for inst in bb.instructions:
            si = inst.sync_info
            waits = list(si.on_wait) if si is not None and si.on_wait else []
            updates = list(si.on_update) if si is not None and si.on_update else []
            if len(waits) > 1:
                for w in waits[:-1]:
                    new.append(
                        mybir.InstNoOp(
                            name=f"I-{nc.next_id()}-waitsplit",
                            engine=inst.engine,
                            ins=[],
                            outs=[],
                            sync_info=mybir.SyncInfo(on_wait=[w], on_update=[]),
                        )
                    )
                si.on_wait = [waits[-1]]
                changed = True
            new.append(inst)
            if len(updates) > 1:
                si.on_update = [updates[0]]
                for u in updates[1:]:
                    new.append(
                        mybir.InstNoOp(
                            name=f"I-{nc.next_id()}-updsplit",
                            engine=inst.engine,
                            ins=[],
                            outs=[],
                            sync_info=mybir.SyncInfo(on_wait=[], on_update=[u]),
                        )
                    )
                changed = True
        if changed:
            while len(bb.instructions):
                bb.instructions.pop()
            for i in new:
                bb.instructions.append(i)

    def walk(bb):
        fix_block(bb)
        for sb in getattr(bb, "blocks", []) or []:
            walk(sb)

    for fn in nc.m.functions:
        for bb in fn.blocks:
            walk(bb)


def _build_program():
    nc = bass.Bass("TRN2", target_bir_lowering=False)

    x = nc.dram_tensor("x", [R, V], f32, kind="ExternalInput")
    protoT = nc.dram_tensor("protoT", [V, NB], f32, kind="ExternalInput")
    gum = nc.dram_tensor("gum", [R, NB], f32, kind="ExternalInput")
    iota1 = nc.dram_tensor("iota1", [P, W], f32, kind="ExternalInput")
    basem1 = nc.dram_tensor("basem1", [P, NCH], f32, kind="ExternalInput")
    out = nc.dram_tensor("out", [R, NB], f32, kind="ExternalOutput")

    with tile.TileContext(nc) as tc:
        with (
            tc.tile_pool(name="const", bufs=1) as constp,
            tc.tile_pool(name="xin", bufs=4) as xp,
            tc.tile_pool(name="small", bufs=3) as sp,
        ):
            iota_t = constp.tile([P, W], f32)
            nc.sync.dma_start(out=iota_t[:, :], in_=iota1[:, :])
            base_t = constp.tile([P, NCH], f32)
            nc.sync.dma_start(out=base_t[:, :], in_=basem1[:, :])

            for r in range(RT):
                rows = slice(r * P, (r + 1) * P)
                accs = sp.tile([P, NCH], f32, name="accs", tag="accs")
                for c in range(NCH):
                    w = min(W, V - c * W)
                    xt = xp.tile([P, W], f32, name="xt", tag="xt")
                    nc.sync.dma_start(
                        out=xt[:, :w], in_=x[rows, c * W : c * W + w]
                    )
                    # in-place multiply by local 1-based iota on VectorE
                    nc.vector.tensor_tensor(
                        out=xt[:, :w],
                        in0=xt[:, :w],
                        in1=iota_t[:, :w],
                        op=mybir.AluOpType.mult,
                    )
                    # row-sum on ScalarE (in-place copy + accumulate):
                    # f_c = 1 + local_idx if the 1.0 is in this chunk, else 0
                    nc.scalar.activation(
                        out=xt[:, :w],
                        in_=xt[:, :w],
                        func=mybir.ActivationFunctionType.Copy,
                        bias=0.0,
                        accum_out=accs[:, c : c + 1],
                    )

                # v* = sum_c (f_c + (c*W - 1)) * min(f_c, 1)
                mask = sp.tile([P, NCH], f32, name="mask", tag="mask")
                nc.vector.tensor_scalar(
                    out=mask[:, :],
                    in0=accs[:, :],
                    scalar1=1.0,
                    scalar2=None,
                    op0=mybir.AluOpType.min,
                )
                shifted = sp.tile([P, NCH], f32, name="shifted", tag="shifted")
                nc.vector.tensor_tensor(
                    out=shifted[:, :],
                    in0=accs[:, :],
                    in1=base_t[:, :],
                    op=mybir.AluOpType.add,
                )
                nc.vector.tensor_tensor(
                    out=shifted[:, :],
                    in0=shifted[:, :],
                    in1=mask[:, :],
                    op=mybir.AluOpType.mult,
                )
                vstar = sp.tile([P, 1], f32, name="vstar", tag="vstar")
                nc.vector.tensor_reduce(
                    out=vstar[:, :],
                    in_=shifted[:, :],
                    axis=mybir.AxisListType.X,
                    op=mybir.AluOpType.add,
                )
                idx = sp.tile([P, 1], mybir.dt.int32, name="idx", tag="idx")
                nc.vector.tensor_copy(out=idx[:, :], in_=vstar[:, :])
                sims = sp.tile([P, NB], f32, name="sims", tag="sims")
                nc.gpsimd.indirect_dma_start(
                    out=sims[:, :],
                    out_offset=None,
                    in_=protoT[:, :],
                    in_offset=bass.IndirectOffsetOnAxis(ap=idx[:, :1], axis=0),
                )
                gt = sp.tile([P, NB], f32, name="gt", tag="gt")
                nc.sync.dma_start(out=gt[:, :], in_=gum[rows, :])

                # z = sims/T + gumbel
                z0 = sp.tile([P, NB], f32, name="z0", tag="z0")
                nc.scalar.mul(out=z0[:, :], in_=sims[:, :], mul=1.0 / TEMPERATURE)
                z = sp.tile([P, NB], f32, name="z", tag="z")
                nc.vector.tensor_tensor(
                    out=z[:, :], in0=z0[:, :], in1=gt[:, :], op=mybir.AluOpType.add
                )
                rmax = sp.tile([P, 1], f32, name="rmax", tag="rmax")
                nc.vector.tensor_reduce(
                    out=rmax[:, :],
                    in_=z[:, :],
                    axis=mybir.AxisListType.X,
                    op=mybir.AluOpType.max,
                )
                # y_hard = (z == rowmax); softmax(z) = exp(z - rowmax)/sum
                yh = sp.tile([P, NB], f32, name="yh", tag="yh")
                nc.vector.tensor_scalar(
                    out=yh[:, :],
                    in0=z[:, :],
                    scalar1=rmax[:, :1],
                    scalar2=None,
                    op0=mybir.AluOpType.is_equal,
                )
                zs = sp.tile([P, NB], f32, name="zs", tag="zs")
                nc.vector.tensor_scalar(
                    out=zs[:, :],
                    in0=z[:, :],
                    scalar1=rmax[:, :1],
                    scalar2=None,
                    op0=mybir.AluOpType.subtract,
                )
                e = sp.tile([P, NB], f32, name="e", tag="e")
                den = sp.tile([P, 1], f32, name="den", tag="den")
                nc.scalar.activation(
                    out=e[:, :],
                    in_=zs[:, :],
                    func=mybir.ActivationFunctionType.Exp,
                    accum_out=den[:, :],
                )
                rden = sp.tile([P, 1], f32, name="rden", tag="rden")
                nc.vector.reciprocal(out=rden[:, :], in_=den[:, :])
                ys = sp.tile([P, NB], f32, name="ys", tag="ys")
                nc.vector.tensor_scalar(
                    out=ys[:, :],
                    in0=e[:, :],
                    scalar1=rden[:, :1],
                    scalar2=None,
                    op0=mybir.AluOpType.mult,
                )
                # straight-through: out = (y_hard - y_soft) + y_soft
                d = sp.tile([P, NB], f32, name="d", tag="d")
                nc.vector.tensor_tensor(
                    out=d[:, :], in0=yh[:, :], in1=ys[:, :], op=mybir.AluOpType.subtract
                )
                o = sp.tile([P, NB], f32, name="o", tag="o")
                nc.vector.tensor_tensor(
                    out=o[:, :], in0=d[:, :], in1=ys[:, :], op=mybir.AluOpType.add
                )
                nc.sync.dma_start(out=out[rows, :], in_=o[:, :])

    _legalize_sync(nc)
    return nc


def _get_program():
    global _PROGRAM
    if _PROGRAM is None:
        _PROGRAM = _build_program()
    return _PROGRAM


def kernel(onehot_tokens, prototypes, gumbel_noise):
    global LAST_RESULT
    X = np.asarray(onehot_tokens, dtype=np.float32).reshape(B * S, V)
    G = np.ascontiguousarray(np.asarray(gumbel_noise, dtype=np.float32)).reshape(
        B * S, NB
    )
    PT = np.ascontiguousarray(np.asarray(prototypes, dtype=np.float32).T)
    iota1 = np.ascontiguousarray(
        np.broadcast_to(np.arange(1, W + 1, dtype=np.float32)[None, :], (P, W))
    )
    basem1 = np.ascontiguousarray(
        np.broadcast_to(
            (np.arange(NCH, dtype=np.float32) * W - 1.0)[None, :], (P, NCH)
        )
    )

    nc = _get_program()
    in_maps = [
        {
            "x": np.ascontiguousarray(X[c * R : (c + 1) * R]),
            "protoT": PT,
            "gum": np.ascontiguousarray(G[c * R : (c + 1) * R]),
            "iota1": iota1,
            "basem1": basem1,
        }
        for c in range(N_CORES)
    ]
    res = run_bass_kernel_spmd(nc, in_maps, core_ids=list(range(N_CORES)), trace=TRACE)
    LAST_RESULT = res
    outs = np.concatenate([res.results[c]["out"] for c in range(N_CORES)], axis=0)
    return outs.reshape(B, S, NB).astype(np.float32)
